# revision 19
# baseline (speedup 1.0000x reference)
"""MoE v5: expert-parallel with output-channel-quarter work units.

Work unit = (expert e, output quarter q) costing count_e token-columns.
32 units are sorted by cost and dealt into 4 uniform slots of 8 (one unit
per core per slot).  Slot s has compile-time capacity C_s = the largest
count in that slot, so per-core work is sum_s C_s columns — [2236, 2092,
2043, 1943] for the seed-0 routing = 8314 cols vs 2*2236=4472*... (v4: 16
o-chunks x 2236).  PE floor drops from 238.5us to 221.7us.

Each unit needs only its expert's quarter-weight [512, 2048] (weight DMA
total unchanged) but its expert's full token set (xt traffic x2 per extra
slot level; ~34MB/core, still far under the compute time at ~360GB/s).
"""

import numpy as np
import ml_dtypes

N, D, E, TOP_K = 8192, 2048, 8, 2
P = 128
KO = D // P      # 16 contraction tiles
NSLOTS = 4
OTS = D // P // NSLOTS  # o-chunks per slot (4 -> 512 output channels)
QW = D // NSLOTS        # output channels per quarter (512)

PROFILE = False
LAST_RESULTS = None

_KERNEL_CACHE = {}


def _routing(x, W_gate, b_gate):
    import jax

    cpu = jax.devices("cpu")[0]
    with jax.default_device(cpu):
        xj = jax.device_put(np.asarray(x, dtype=np.float32), cpu)
        wg = jax.device_put(np.asarray(W_gate, dtype=np.float32), cpu)
        bg = jax.device_put(np.asarray(b_gate, dtype=np.float32), cpu)
        logits = xj @ wg.T + bg
        gate = jax.nn.softmax(logits, axis=-1)
        vals, idx = jax.lax.top_k(gate, TOP_K)
        vals, idx = np.asarray(vals), np.asarray(idx)
    return vals, idx


def _ctiles(C):
    # keep matmul free dims >=256 where possible (LDWEIGHTS-bound below that)
    widths = []
    rem = C
    while rem > 1024:
        widths.append(512)
        rem -= 512
    if rem > 512:
        widths.extend([(rem + 1) // 2, rem // 2])
    elif rem:
        widths.append(rem)
    tiles, c0 = [], 0
    for w in widths:
        tiles.append((c0, w))
        c0 += w
    return tuple(tiles)


def _build(caps):
    import concourse.tile as tile
    from concourse import bacc, mybir

    nc = bacc.Bacc("TRN2", target_bir_lowering=False, debug=False)
    wt, xt, bias, yt, ctl = [], [], [], [], []
    for s, C in enumerate(caps):
        wt.append(
            nc.dram_tensor(
                f"wt{s}", [OTS, P, KO, P], mybir.dt.bfloat16, kind="ExternalInput"
            ).ap()
        )
        xt.append(
            nc.dram_tensor(
                f"xt{s}", [P, KO, C], mybir.dt.bfloat16, kind="ExternalInput"
            ).ap()
        )
        bias.append(
            nc.dram_tensor(
                f"bias{s}", [P, OTS], mybir.dt.float32, kind="ExternalInput"
            ).ap()
        )
        yt.append(
            nc.dram_tensor(
                f"yt{s}", [P, OTS, C], mybir.dt.bfloat16, kind="ExternalOutput"
            ).ap()
        )
        ctl.append(_ctiles(C))

    with tile.TileContext(nc) as tc:
        with (
            tc.tile_pool(name="consts", bufs=1) as cpool,
            tc.tile_pool(name="xchunks", bufs=4) as xpool,
            tc.tile_pool(name="outs", bufs=12) as opool,
            tc.tile_pool(name="psum", bufs=8, space="PSUM") as pspool,
        ):
            # PE clock warm-up: the HAM gate keeps the PE at 1.2GHz until
            # it sees ~3.4us of sustained activity, and the kernel head is
            # DMA-bound anyway — burn the wait on scratch matmuls so the
            # real matmuls start at the full 2.4GHz (the profile showed
            # ~15.6us of throttled-active PE time without this).
            warm = cpool.tile([P, 512], mybir.dt.bfloat16, name="warm")
            nc.vector.memset(warm[:], 0.0)

            def warmup(n):
                wps = pspool.tile([P, 512], mybir.dt.float32, tag="ps")
                for _ in range(n):
                    nc.tensor.matmul(
                        wps[:], warm[:, :P], warm[:], start=True, stop=True
                    )
                # consume the result so it can't be dead-code-eliminated
                nc.vector.tensor_copy(warm[:1, :4], wps[:1, :4])

            warmup(64)

            wt_sb = [cpool.tile([P, OTS, KO, P], mybir.dt.bfloat16, name=f"w{s}")
                     for s in range(NSLOTS)]
            bias_sb = [cpool.tile([P, OTS], mybir.dt.float32, name=f"b{s}")
                       for s in range(NSLOTS)]
            # xt chunks stream through a shared pool (uniform max-size tiles)
            maxcw = max(cw for tiles in ctl for _, cw in tiles)
            xt_sb = {}  # (s, ci) -> (tile, cw)

            def load_xt(s, ci, engine, ksplit=1):
                c0, cw = ctl[s][ci]
                t = xpool.tile([P, KO, maxcw], mybir.dt.bfloat16, tag="xc")
                xt_sb[(s, ci)] = t
                kc = KO // ksplit
                for i in range(ksplit):
                    engine.dma_start(
                        t[:, i * kc : (i + 1) * kc, :cw],
                        xt[s][:, i * kc : (i + 1) * kc, c0 : c0 + cw],
                    )
                return t

            def load_wt(s, o, ksplit=1):
                kc = KO // ksplit
                for i in range(ksplit):
                    nc.scalar.dma_start(
                        wt_sb[s][:, o, i * kc : (i + 1) * kc],
                        wt[s][o, :, i * kc : (i + 1) * kc],
                    )

            # Head-critical loads first, then steady-state in consumption
            # order.  sync queue: first two xt chunks of slot 0 + output
            # stores; scalar queue: all weights/bias, remaining xt chunks.
            for o in range(OTS):
                load_wt(0, o, ksplit=2 if o == 0 else 1)
            load_xt(0, 0, nc.sync, ksplit=2)
            nc.scalar.dma_start(bias_sb[0][:], bias[0][:])
            if len(ctl[0]) > 1:
                load_xt(0, 1, nc.scalar)
            for o in range(OTS):
                load_wt(1, o)
            nc.scalar.dma_start(bias_sb[1][:], bias[1][:])
            for ci in range(2, len(ctl[0])):
                load_xt(0, ci, nc.scalar)
            for s in (2, 3):
                for o in range(OTS):
                    load_wt(s, o)
                nc.scalar.dma_start(bias_sb[s][:], bias[s][:])
            for s in (1, 2, 3):
                for ci in range(len(ctl[s])):
                    load_xt(s, ci, nc.scalar)

            group = 0
            for s in range(NSLOTS):
                for ci, (c0, cw) in enumerate(ctl[s]):
                    if s == 0 and ci == 1:
                        # the head is HBM-bound: the PE idles here waiting on
                        # early xt/weight bytes and would re-throttle to
                        # 1.2GHz after ~3.4us — keep it busy on scratch work
                        warmup(16)
                    xtile = xt_sb[(s, ci)]
                    for o in range(OTS):
                        ps = pspool.tile([P, 512], mybir.dt.float32)
                        for ko in range(KO):
                            nc.tensor.matmul(
                                ps[:, :cw],
                                wt_sb[s][:, o, ko],
                                xtile[:, ko, :cw],
                                start=(ko == 0),
                                stop=(ko == KO - 1),
                            )
                        ot = opool.tile([P, 512], mybir.dt.bfloat16)
                        nc.vector.tensor_scalar(
                            ot[:, :cw],
                            ps[:, :cw],
                            bias_sb[s][:, o : o + 1],
                            0.0,
                            mybir.AluOpType.add,
                            mybir.AluOpType.max,
                        )
                        # first 12 stores stay on sync: the scalar queue is
                        # still draining input bytes early on, and a store
                        # stuck behind them would delay out-slot recycling
                        out_eng = (
                            nc.sync if (group < 12 or group % 2 == 0) else nc.scalar
                        )
                        out_eng.dma_start(yt[s][:, o, c0 : c0 + cw], ot[:, :cw])
                        group += 1
    nc.compile()
    return nc


def _get_kernel(caps):
    if caps not in _KERNEL_CACHE:
        _KERNEL_CACHE[caps] = _build(caps)
    return _KERNEL_CACHE[caps]


def kernel(x, W_experts, b_experts, W_gate, b_gate):
    global LAST_RESULTS
    x = np.asarray(x, dtype=np.float32)
    W_experts = np.asarray(W_experts, dtype=np.float32)
    b_experts = np.asarray(b_experts, dtype=np.float32)

    vals, idx = _routing(x, W_gate, b_gate)

    sels, combine_w, counts = [], [], []
    for e in range(E):
        mask = idx == e
        rows = np.nonzero(mask.any(axis=1))[0]
        sels.append(rows)
        combine_w.append(vals[mask])
        counts.append(len(rows))

    # 32 units -> 4 slots of 8 (one per core), sorted by cost
    units = sorted(
        ((counts[e], e, q) for e in range(E) for q in range(NSLOTS)),
        key=lambda t: (-t[0], t[1], t[2]),
    )
    slots = [units[8 * s : 8 * s + 8] for s in range(NSLOTS)]
    caps = tuple(max(4, ((sl[0][0] + 3) // 4) * 4) for sl in slots)
    nc = _get_kernel(caps)

    xbf = np.ascontiguousarray(x.astype(ml_dtypes.bfloat16))
    # pack each expert's tokens once: [P, KO, cnt]
    xpack = []
    for e in range(E):
        cnt = counts[e]
        xsel = xbf[sels[e]]
        xpack.append(
            np.ascontiguousarray(xsel.T.reshape(KO, P, cnt).transpose(1, 0, 2))
        )

    in_maps = [{} for _ in range(E)]
    for s in range(NSLOTS):
        C = caps[s]
        for i in range(8):
            cnt, e, q = slots[s][i]
            xe = np.zeros((P, KO, C), dtype=ml_dtypes.bfloat16)
            xe[:, :, :cnt] = xpack[e]
            we = np.ascontiguousarray(
                W_experts[e][q * QW : (q + 1) * QW]
                .astype(ml_dtypes.bfloat16)
                .reshape(OTS, P, KO, P)
                .transpose(0, 3, 2, 1)
            )
            be = np.ascontiguousarray(
                b_experts[e][q * QW : (q + 1) * QW].reshape(OTS, P).T
            )
            in_maps[i][f"wt{s}"] = we
            in_maps[i][f"xt{s}"] = xe
            in_maps[i][f"bias{s}"] = be

    from concourse.bass_utils import run_bass_kernel_spmd

    res = run_bass_kernel_spmd(nc, in_maps, core_ids=list(range(E)), trace=PROFILE)
    LAST_RESULTS = res

    out = np.zeros((N, D), dtype=np.float32)
    for s in range(NSLOTS):
        for i in range(8):
            cnt, e, q = slots[s][i]
            yt_si = res.results[i][f"yt{s}"]  # [P, OTS, C] bf16
            y = (
                yt_si[:, :, :cnt]
                .astype(np.float32)
                .transpose(2, 1, 0)
                .reshape(cnt, QW)
            )
            out[sels[e], q * QW : (q + 1) * QW] += combine_w[e][:, None] * y
    return out
